# revision 1
# baseline (speedup 1.0000x reference)
"""Trainium2 Bass kernel: nn_CorrBlockSingleScale (RAFT single-scale correlation lookup).

reference: corr[b,n] = fmap1[b,:,n] . fmap2[b,:,m] / 16 as a [HW, H, W] volume;
out[b, k1*9+k2, h, w] = bilinear(corr[b,(h,w)], x=cx+k1-4, y=cy+k2-4), zeros padding.

Sharding: data-parallel over the B*H*W = 8192 pixel axis; core c handles batch
c//4, pixels (c%4)*1024 ... +1024 (8 blocks of 128 pixels). No cross-core comms.

Per 128-pixel block on each core:
  PE      : C[128pix, 4096] = f1_blk^T @ f2 (fp32r, N=512 chunks, K=256)
  DVE/ACT : PSUM -> SBUF copy (split engines)
  DMA     : C -> DRAM table [pix, 65, 64] (row 64 stays zero; y-OOB taps index it)
  gather  : dma_gather 10 map rows/pixel (256B rows) -> G[128, 10, 64]
  DVE/ACT : y-lerp (per-partition u), 6-stage binary select ladder for the
            per-pixel x-window shift (bits of floor(cx) as masks), x-lerp (v)
  DMA     : O[128, 81] -> out
Host: index/weight precompute from coords, final reshape/transpose.
"""

import numpy as np

import concourse.bass as bass
import concourse.mybir as mybir
import concourse.tile as tile
from concourse import bacc, library_config
from concourse.bass_utils import run_bass_kernel_spmd

F32 = mybir.dt.float32
F32R = mybir.dt.float32r
I16 = mybir.dt.int16
COPY = mybir.ActivationFunctionType.Copy
MULT = mybir.AluOpType.mult

NCORES = 8
NPC = 1024          # pixels per core
NBLK = 8            # blocks of 128 pixels per core
ROWS = 65           # 64 data rows + 1 zero row per pixel in the DRAM table
WPAD = 76           # padded row width for the shift ladder (4 + 64 + 5 + align)
STAGES = [(32, 41), (16, 25), (8, 17), (4, 13), (2, 11), (1, 10)]

_NC = None


def _s(st, b, j):
    """[128,1] per-partition scalar view of scal tile column j for block b."""
    return st[:, b : b + 1, j : j + 1].rearrange("p a c -> p (a c)")


def _build_kernel(tc, out, corr, f1, f2, idx, scal):
    nc = tc.nc
    import contextlib

    with contextlib.ExitStack() as ctx:
        const = ctx.enter_context(tc.tile_pool(name="const", bufs=1))
        work = ctx.enter_context(tc.tile_pool(name="work", bufs=3))
        cpool = ctx.enter_context(tc.tile_pool(name="cpool", bufs=2))
        psum = ctx.enter_context(tc.tile_pool(name="psum", bufs=8, space="PSUM"))

        nc.gpsimd.load_library(library_config.mlp)

        f1t = const.tile([128, 2, NPC], F32R)
        nc.sync.dma_start(f1t[:], f1[:])
        f2t = const.tile([128, 2, 4096], F32R)
        nc.sync.dma_start(f2t[:], f2[:])
        idxt = const.tile([128, NBLK, 80], I16)
        nc.sync.dma_start(idxt[:], idx[:])
        st = const.tile([128, NBLK, 16], F32)
        nc.sync.dma_start(st[:], scal[:])

        # zero row 64 of every pixel's table (y-OOB taps gather from it)
        zt = const.tile([128, 64], F32)
        nc.vector.memset(zt[:], 0.0)
        for b in range(NBLK):
            nc.sync.dma_start(
                corr[b * 128 : (b + 1) * 128, 64:65, :],
                zt[:].rearrange("p (a c) -> p a c", a=1),
            )

        # persistent padded tiles; pad columns stay zero across blocks
        xps = [
            const.tile([128, 9, WPAD], F32, name=f"xp{i}", tag=f"xp{i}")
            for i in range(2)
        ]
        nc.vector.memset(xps[0][:], 0.0)
        nc.vector.memset(xps[1][:], 0.0)

        for b in range(NBLK):
            pix = slice(b * 128, (b + 1) * 128)
            # ---- matmul: C = f1_blk^T @ f2  (K=256 as 2x128, N=4096 as 8x512)
            lhs = [
                f1t[:, k : k + 1, b * 128 : (b + 1) * 128]
                .rearrange("p a c -> p (a c)")
                for k in range(2)
            ]
            pts = []
            for n in range(8):
                pt = psum.tile([128, 512], F32, tag="ps")
                pts.append(pt)
                for k in range(2):
                    nc.tensor.matmul(
                        pt[:],
                        lhsT=lhs[k],
                        rhs=f2t[:, k : k + 1, n * 512 : (n + 1) * 512]
                        .rearrange("p a c -> p (a c)"),
                        start=(k == 0),
                        stop=(k == 1),
                    )
            ct = cpool.tile([128, 4096], F32, tag="C")
            for n in range(8):
                dst = ct[:, n * 512 : (n + 1) * 512]
                if n % 2 == 0:
                    nc.vector.tensor_copy(dst, pts[n][:])
                else:
                    nc.scalar.copy(dst, pts[n][:])
            # ---- write corr block to the DRAM table (rows 0..63)
            nc.sync.dma_start(
                corr[pix, 0:64, :], ct[:].rearrange("p (a c) -> p a c", a=64)
            )
            # ---- gather 10 rows per pixel
            gt = work.tile([128, 10, 64], F32, tag="G")
            table = corr[pix].rearrange("a b c -> (a b) c")
            nc.gpsimd.dma_gather(
                gt[:],
                table,
                idxt[:, b : b + 1, :].rearrange("p a c -> p (a c)"),
                1280,
                1280,
                64,
                single_packet=False,
            )
            # ---- y-lerp into the padded tile: xp[:,:,4:68] = (1-u)G0 + u*G1
            xp = xps[b % 2]
            t0 = work.tile([128, 9, 64], F32, tag="T0")
            nc.scalar.activation(t0[:], gt[:, 0:9, :], COPY, scale=_s(st, b, 0))
            nc.vector.tensor_scalar(
                xp[:, :, 4:68], gt[:, 1:10, :], _s(st, b, 1), None, MULT
            )
            nc.vector.tensor_add(xp[:, :, 4:68], xp[:, :, 4:68], t0[:])
            # ---- 6-stage binary select ladder: shift by sx = floor(cx) in [0,63]
            cur = xp[:, :, 0:73]
            for k, (sh, wn) in enumerate(STAGES):
                nxt = work.tile([128, 9, wn], F32, tag=f"L{k}")
                mask = (
                    _s(st, b, 4 + k)
                    .bitcast(mybir.dt.int32)
                    .to_broadcast([128, 9, wn])
                )
                nc.scalar.copy(nxt[:], cur[:, :, 0:wn])
                nc.vector.copy_predicated(nxt[:], mask, cur[:, :, sh : sh + wn])
                cur = nxt
            # ---- x-lerp: O = (1-v)X[0:9] + v*X[1:10]
            t1 = work.tile([128, 9, 9], F32, tag="T1")
            nc.scalar.activation(t1[:], cur[:, :, 0:9], COPY, scale=_s(st, b, 2))
            ot = work.tile([128, 9, 9], F32, tag="OT")
            nc.vector.tensor_scalar(ot[:], cur[:, :, 1:10], _s(st, b, 3), None, MULT)
            nc.vector.tensor_add(ot[:], ot[:], t1[:])
            nc.sync.dma_start(out[pix, :], ot[:].rearrange("p a c -> p (a c)"))


def _build():
    nc = bacc.Bacc("TRN2", target_bir_lowering=False, debug=False)
    f1 = nc.dram_tensor("f1", [128, 2, NPC], F32R, kind="ExternalInput").ap()
    f2 = nc.dram_tensor("f2", [128, 2, 4096], F32R, kind="ExternalInput").ap()
    idx = nc.dram_tensor("idx", [128, NBLK, 80], I16, kind="ExternalInput").ap()
    scal = nc.dram_tensor("scal", [128, NBLK, 16], F32, kind="ExternalInput").ap()
    out = nc.dram_tensor("out", [NPC, 81], F32, kind="ExternalOutput").ap()
    corr = nc.dram_tensor("corr", [NPC, ROWS, 64], F32, kind="Internal").ap()
    with tile.TileContext(nc) as tc:
        _build_kernel(tc, out, corr, f1, f2, idx, scal)
    nc.compile()
    return nc


def get_nc():
    global _NC
    if _NC is None:
        _NC = _build()
    return _NC


def host_prep(fmap1, fmap2, coords, radius):
    """Per-core input maps. All coord-derived indices/weights computed here."""
    B, D, H, W = fmap1.shape
    assert (B, D, H, W) == (2, 256, 64, 64) and int(radius) == 4
    f1 = (fmap1.reshape(B, D, H * W) / np.float32(16.0)).astype(np.float32)
    f2 = fmap2.reshape(B, D, H * W).astype(np.float32)
    cx = coords[:, 0].reshape(B, H * W).astype(np.float32)
    cy = coords[:, 1].reshape(B, H * W).astype(np.float32)

    in_maps = []
    for c in range(NCORES):
        b, ps = c // 4, (c % 4) * NPC
        f1c = np.ascontiguousarray(
            f1[b, :, ps : ps + NPC].reshape(2, 128, NPC).transpose(1, 0, 2)
        )
        f2c = np.ascontiguousarray(f2[b].reshape(2, 128, 4096).transpose(1, 0, 2))
        ccx = cx[b, ps : ps + NPC]
        ccy = cy[b, ps : ps + NPC]
        x0 = np.floor(ccx)
        y0 = np.floor(ccy)
        v = ccx - x0  # frac x
        u = ccy - y0  # frac y
        sx = x0.astype(np.int64)  # in [0, 63]
        y0i = y0.astype(np.int64)

        # gather indices: [8 blocks][1280] -> wrapped [128, 80] int16
        idxc = np.zeros((128, NBLK, 80), np.int16)
        scalc = np.zeros((128, NBLK, 16), np.float32)
        for blk in range(NBLK):
            p = np.arange(128)
            gp = blk * 128 + p
            rows = y0i[gp][None, :] - 4 + np.arange(10)[:, None]  # [10, 128]
            rows = np.where((rows < 0) | (rows > 63), 64, rows)
            ilist = (p[None, :] * ROWS + rows).reshape(1280).astype(np.int16)
            idxc[:, blk, :] = np.tile(ilist.reshape(80, 16).T, (8, 1))
            scalc[:, blk, 0] = 1.0 - u[gp]
            scalc[:, blk, 1] = u[gp]
            scalc[:, blk, 2] = 1.0 - v[gp]
            scalc[:, blk, 3] = v[gp]
            for k, (sh, _) in enumerate(STAGES):
                scalc[:, blk, 4 + k] = ((sx[gp] // sh) % 2).astype(np.float32)
        in_maps.append(
            {
                "f1": f1c,
                "f2": f2c,
                "idx": np.ascontiguousarray(idxc),
                "scal": np.ascontiguousarray(scalc),
            }
        )
    return in_maps


def assemble(outs):
    """8x [1024, 81] (k2-major within 81) -> [2, 81, 64, 64] with k = k1*9+k2."""
    o = np.stack(outs).reshape(2, 4096, 81).reshape(2, 64, 64, 9, 9)
    return np.ascontiguousarray(
        o.transpose(0, 4, 3, 1, 2).reshape(2, 81, 64, 64)
    ).astype(np.float32)


def kernel(**inputs):
    fmap1 = np.asarray(inputs["fmap1"], np.float32)
    fmap2 = np.asarray(inputs["fmap2"], np.float32)
    coords = np.asarray(inputs["coords"], np.float32)
    radius = int(np.asarray(inputs["radius"]))
    in_maps = host_prep(fmap1, fmap2, coords, radius)
    nc = get_nc()
    res = run_bass_kernel_spmd(nc, in_maps, core_ids=list(range(NCORES)))
    return assemble([r["out"] for r in res.results])



# revision 6
# speedup vs baseline: 1.2382x; 1.2382x over previous
"""Trainium2 Bass kernel: nn_CorrBlockSingleScale (RAFT single-scale correlation lookup).

reference: corr[b,n] = fmap1[b,:,n] . fmap2[b,:,m] / 16 as a [HW, H, W] volume;
out[b, k1*9+k2, h, w] = bilinear(corr[b,(h,w)], x=cx+k1-4, y=cy+k2-4), zeros padding.

Sharding: data-parallel over the B*H*W = 8192 pixel axis; core c handles batch
c//4, pixels (c%4)*1024 ... +1024.

Key ideas vs the all-pairs formulation:
 - pixels are SORTED by floor(cy) on the host, so each 128-pixel block only
   needs a ~22-row window of its 64x64 correlation map: the matmul computes
   just that window and it stays in SBUF (no DRAM corr table, no dma_gather).
 - the per-pixel row selection is a Pool-engine indirect_copy (per-16-group
   base rows) followed by a tiny fp16 residual shift ladder.
 - everything downstream of PSUM runs in fp16 (tolerance is 2e-2).

Per 128-pixel block on each core:
  PE      : W[128pix, 22*64] = f1_blk^T @ f2win_b (fp16 in, f32 psum)
  ACT/DVE : PSUM -> SBUF fp16 window
  Pool    : indirect_copy 13 rows/group-of-16 (per-group base from SBUF idx)
  DVE     : 2-stage residual y shift ladder, y-lerp
  DVE/Pool: x shift ladder (6 stages, sx = floor(cx)), x-lerp
  DMA     : O[128, 81] f32 -> out
Host: sort, f2 window gather, weight/mask/idx precompute, unsort+transpose.
"""

import numpy as np

import concourse.bass as bass
import concourse.mybir as mybir
import concourse.tile as tile
from concourse import bacc
from concourse.bass_utils import run_bass_kernel_spmd

F32 = mybir.dt.float32
F16 = mybir.dt.float16
U16 = mybir.dt.uint16
I16 = mybir.dt.int16
COPY = mybir.ActivationFunctionType.Copy
MULT = mybir.AluOpType.mult

NCORES = 8
NPC = 1024          # pixels per core
NBLK = 8            # blocks of 128 pixels per core
SROW = 22           # f2 window rows per block (block y-span + 10 <= 22)
NGR = 13            # rows gathered per group of 16 pixels (group span + 10 <= 13)
YSTAGES = [(2, 11), (1, 10)]
XSTAGES = [(32, 41), (16, 25), (8, 17), (4, 13), (2, 11), (1, 10)]
WPAD = 76           # padded row width for the x shift ladder (4 + 64 + 5 + align)

_NC = None


def _sc(st, b, j):
    """[128,1] per-partition scalar view of tile column j, block b."""
    return st[:, b : b + 1, j : j + 1].rearrange("p a c -> p (a c)")


def _build_kernel(tc, out, f1, f2w, wuv, msk, gidx):
    nc = tc.nc
    import contextlib

    with contextlib.ExitStack() as ctx:
        const = ctx.enter_context(tc.tile_pool(name="const", bufs=1))
        work = ctx.enter_context(tc.tile_pool(name="work", bufs=3))
        psum = ctx.enter_context(tc.tile_pool(name="psum", bufs=2, space="PSUM"))

        f1t = const.tile([128, 2, NPC], F16)
        nc.sync.dma_start(f1t[:], f1[:])
        f2t = const.tile([128, 2, NBLK, SROW * 64], F16)
        nc.sync.dma_start(f2t[:], f2w[:])
        wt_uv = const.tile([128, NBLK, 4], F32)
        nc.sync.dma_start(wt_uv[:], wuv[:])
        mt = const.tile([128, NBLK, 12], F16)
        nc.sync.dma_start(mt[:], msk[:])
        it = const.tile([128, NBLK, 4], U16)
        nc.sync.dma_start(it[:], gidx[:])

        # persistent padded tiles for the x ladder; pad columns stay zero
        xps = [
            const.tile([128, 9, WPAD], F16, name=f"xp{i}", tag=f"xp{i}")
            for i in range(2)
        ]
        nc.vector.memset(xps[0][:], 0.0)
        nc.vector.memset(xps[1][:], 0.0)

        for b in range(NBLK):
            pix = slice(b * 128, (b + 1) * 128)
            # ---- matmul: W = f1_blk^T @ f2win  (K=256 as 2x128)
            pt = psum.tile([128, SROW * 64], F32, tag="ps")
            lhs = [
                f1t[:, k : k + 1, b * 128 : (b + 1) * 128].rearrange(
                    "p a c -> p (a c)"
                )
                for k in range(2)
            ]
            rhs = [
                f2t[:, k : k + 1, b : b + 1, :].rearrange("p a b c -> p (a b c)")
                for k in range(2)
            ]
            for n0 in range(0, SROW * 64, 512):
                n1 = min(n0 + 512, SROW * 64)
                for k in range(2):
                    nc.tensor.matmul(
                        pt[:, n0:n1],
                        lhsT=lhs[k],
                        rhs=rhs[k][:, n0:n1],
                        start=(k == 0),
                        stop=(k == 1),
                    )
            # ---- PSUM -> SBUF fp16 window
            wt = work.tile([128, SROW, 64], F16, tag="W")
            ptv = pt[:].rearrange("p (r c) -> p r c", c=64)
            nc.scalar.copy(wt[:, 0:11, :], ptv[:, 0:11, :])
            nc.vector.tensor_copy(wt[:, 11:SROW, :], ptv[:, 11:SROW, :])
            # ---- per-16-group row gather: 13 rows as 52 16-elem chunks
            # (IndirectCopy ISA wants inner size 16)
            gt = work.tile([128, NGR, 64], F16, tag="G")
            nc.gpsimd.indirect_copy(
                gt[:].rearrange("p r (k c) -> p (r k) c", c=16),
                wt[:].rearrange("p a (b c) -> p (a b) c", c=16),
                it[:, b, :],
                True,
            )
            # ---- residual y shift ladder: shift by ry in [0, 3]
            cur = gt
            for i, (sh, wn) in enumerate(YSTAGES):
                nxt = work.tile([128, wn, 64], F16, tag=f"Y{i}")
                mask = _sc(mt, b, i).bitcast(I16).to_broadcast([128, wn, 64])
                if i == 0:
                    nc.vector.tensor_copy(nxt[:], cur[:, 0:wn, :])
                else:
                    nc.gpsimd.tensor_copy(nxt[:], cur[:, 0:wn, :])
                nc.vector.copy_predicated(nxt[:], mask, cur[:, sh : sh + wn, :])
                cur = nxt
            # ---- y-lerp into the padded tile: xp[:,:,4:68] = (1-u)Y0 + u*Y1
            xp = xps[b % 2]
            t0 = work.tile([128, 9, 64], F16, tag="T0")
            nc.scalar.activation(t0[:], cur[:, 0:9, :], COPY, scale=_sc(wt_uv, b, 0))
            nc.vector.tensor_scalar(
                xp[:, :, 4:68], cur[:, 1:10, :], _sc(wt_uv, b, 1), None, MULT
            )
            nc.vector.tensor_add(xp[:, :, 4:68], xp[:, :, 4:68], t0[:])
            # ---- x shift ladder: cols <- cols shifted by sx = floor(cx) in [0,63]
            xcur = xp[:, :, 0:73]
            for i, (sh, wn) in enumerate(XSTAGES):
                nxt = work.tile([128, 9, wn], F16, tag=f"L{i}")
                mask = _sc(mt, b, 2 + i).bitcast(I16).to_broadcast([128, 9, wn])
                if i < 3:
                    nc.gpsimd.tensor_copy(nxt[:], xcur[:, :, 0:wn])
                else:
                    nc.vector.tensor_copy(nxt[:], xcur[:, :, 0:wn])
                nc.vector.copy_predicated(nxt[:], mask, xcur[:, :, sh : sh + wn])
                xcur = nxt
            # ---- x-lerp: O = (1-v)X[0:9] + v*X[1:10]
            t1 = work.tile([128, 9, 9], F16, tag="T1")
            nc.scalar.activation(t1[:], xcur[:, :, 0:9], COPY, scale=_sc(wt_uv, b, 2))
            ot = work.tile([128, 9, 9], F32, tag="OT")
            nc.vector.tensor_scalar(
                ot[:], xcur[:, :, 1:10], _sc(wt_uv, b, 3), None, MULT
            )
            nc.vector.tensor_add(ot[:], ot[:], t1[:])
            nc.sync.dma_start(out[pix, :], ot[:].rearrange("p a c -> p (a c)"))


def _build():
    nc = bacc.Bacc("TRN2", target_bir_lowering=False, debug=False)
    f1 = nc.dram_tensor("f1", [128, 2, NPC], F16, kind="ExternalInput").ap()
    f2w = nc.dram_tensor(
        "f2w", [128, 2, NBLK, SROW * 64], F16, kind="ExternalInput"
    ).ap()
    wuv = nc.dram_tensor("wuv", [128, NBLK, 4], F32, kind="ExternalInput").ap()
    msk = nc.dram_tensor("msk", [128, NBLK, 12], F16, kind="ExternalInput").ap()
    gidx = nc.dram_tensor("gidx", [128, NBLK, 4], U16, kind="ExternalInput").ap()
    out = nc.dram_tensor("out", [NPC, 81], F32, kind="ExternalOutput").ap()
    with tile.TileContext(nc) as tc:
        _build_kernel(tc, out, f1, f2w, wuv, msk, gidx)
    nc.compile()
    return nc


def get_nc():
    global _NC
    if _NC is None:
        _NC = _build()
    return _NC


def host_prep(fmap1, fmap2, coords, radius):
    """Per-core input maps. Sorting, window gather, and weights on host."""
    B, D, H, W = fmap1.shape
    assert (B, D, H, W) == (2, 256, 64, 64) and int(radius) == 4
    f1 = (fmap1.reshape(B, D, H * W) / np.float32(16.0)).astype(np.float16)
    f2 = fmap2.reshape(B, D, H, W).astype(np.float16)
    # zero-padded rows: r' = r + 4; extra top slack so base = min(y0) always
    f2p = np.zeros((B, 2, 128, 85, 64), np.float16)
    f2p[:, :, :, 4:68, :] = f2.reshape(B, 2, 128, 64, 64)
    cx = coords[:, 0].reshape(B, H * W).astype(np.float32)
    cy = coords[:, 1].reshape(B, H * W).astype(np.float32)

    in_maps = []
    perms = []
    for c in range(NCORES):
        bb, ps = c // 4, (c % 4) * NPC
        ccx = cx[bb, ps : ps + NPC]
        ccy = cy[bb, ps : ps + NPC]
        y0 = np.floor(ccy).astype(np.int64)  # [0, 63]
        order = np.argsort(y0, kind="stable")
        perms.append(order)
        y0s = y0[order]
        x0s = np.floor(ccx[order]).astype(np.int64)
        us = (ccy[order] - y0s).astype(np.float32)
        vs = (ccx[order] - x0s).astype(np.float32)

        # per-block window bases (padded-row coords), per-group gather bases
        yb = y0s.reshape(NBLK, 128)
        base = yb.min(axis=1)                          # [NBLK], window always fits
        sy = yb - base[:, None]                        # [NBLK, 128] in [0, 12]
        assert sy.min() >= 0 and sy.max() <= SROW - 10, (
            f"block y-span too large: {sy.max()}"
        )
        gmin = sy.reshape(NBLK, 8, 16).min(axis=2)     # [NBLK, 8] group base
        assert gmin.max() <= SROW - NGR, f"group base too large: {gmin.max()}"
        ry = sy - np.repeat(gmin, 16, axis=1)          # residual in [0, 3]
        assert ry.max() <= 3, f"group y-span too large: {ry.max()}"

        # f2 windows: [128(K), 2(kchunk), NBLK, SROW*64]
        f2wc = np.empty((2, 128, NBLK, SROW * 64), np.float16)
        for blk in range(NBLK):
            bs = int(base[blk])
            f2wc[:, :, blk, :] = f2p[bb, :, :, bs : bs + SROW, :].reshape(
                2, 128, SROW * 64
            )
        f2wc = np.ascontiguousarray(f2wc.transpose(1, 0, 2, 3))

        # f1 sorted columns: [128(K), 2(kchunk), NPC]
        f1c = np.ascontiguousarray(
            f1[bb][:, ps + order].reshape(2, 128, NPC).transpose(1, 0, 2)
        )

        # per-pixel weights (f32) and ladder masks (f16), [128, NBLK, .]
        wuvc = np.zeros((128, NBLK, 4), np.float32)
        wuvc[:, :, 0] = (1.0 - us).reshape(NBLK, 128).T
        wuvc[:, :, 1] = us.reshape(NBLK, 128).T
        wuvc[:, :, 2] = (1.0 - vs).reshape(NBLK, 128).T
        wuvc[:, :, 3] = vs.reshape(NBLK, 128).T
        mskc = np.zeros((128, NBLK, 12), np.float16)
        r = ry.copy()
        for i, (sh, _) in enumerate(YSTAGES):
            bit = (r >= sh).astype(np.int64)
            r = r - bit * sh
            mskc[:, :, i] = bit.astype(np.float16).T
        sx = x0s.reshape(NBLK, 128)
        for i, (sh, _) in enumerate(XSTAGES):
            mskc[:, :, 2 + i] = ((sx // sh) % 2).astype(np.float16).T
        # gather indices: unwrapped chunk i = s*16 + j -> row i//4, subcol i%4;
        # stored at partition 16g+j, col s. value = (gmin+row)*64 + (i%4)*16
        gidxc = np.zeros((128, NBLK, 4), np.uint16)
        jj = np.tile(np.arange(16), 8)                  # partition j within group
        gg = np.repeat(np.arange(8), 16)                # group id per partition
        for s in range(4):
            i = s * 16 + jj
            row = np.minimum(i // 4, SROW - 1)
            for blk in range(NBLK):
                r = np.minimum(gmin[blk, gg] + row, SROW - 1)
                gidxc[:, blk, s] = (r * 64 + (i % 4) * 16).astype(np.uint16)
        in_maps.append(
            {
                "f1": f1c,
                "f2w": f2wc,
                "wuv": wuvc,
                "msk": np.ascontiguousarray(mskc),
                "gidx": gidxc,
            }
        )
    return in_maps, perms


def assemble(outs, perms):
    """8x [1024, 81] (sorted pixels, k2-major) -> [2, 81, 64, 64], k = k1*9+k2."""
    full = np.empty((NCORES, NPC, 81), np.float32)
    for c in range(NCORES):
        full[c, perms[c]] = outs[c]
    o = full.reshape(2, 4096, 81).reshape(2, 64, 64, 9, 9)
    return np.ascontiguousarray(
        o.transpose(0, 4, 3, 1, 2).reshape(2, 81, 64, 64)
    ).astype(np.float32)


def kernel(**inputs):
    fmap1 = np.asarray(inputs["fmap1"], np.float32)
    fmap2 = np.asarray(inputs["fmap2"], np.float32)
    coords = np.asarray(inputs["coords"], np.float32)
    radius = int(np.asarray(inputs["radius"]))
    in_maps, perms = host_prep(fmap1, fmap2, coords, radius)
    nc = get_nc()
    res = run_bass_kernel_spmd(nc, in_maps, core_ids=list(range(NCORES)))
    return assemble([r["out"] for r in res.results], perms)


# revision 9
# speedup vs baseline: 1.7476x; 1.4114x over previous
"""Trainium2 Bass kernel: nn_CorrBlockSingleScale (RAFT single-scale correlation lookup).

reference: corr[b,n] = fmap1[b,:,n] . fmap2[b,:,m] / 16 as a [HW, H, W] volume;
out[b, k1*9+k2, h, w] = bilinear(corr[b,(h,w)], x=cx+k1-4, y=cy+k2-4), zeros padding.

Sharding: data-parallel over the B*H*W = 8192 pixel axis; core c handles batch
c//4, pixels (c%4)*1024 ... +1024.

Key ideas vs the all-pairs formulation:
 - pixels are SORTED by floor(cy) on the host, so each 128-pixel block only
   needs a 22-row window of its 64x64 correlation map: the matmul computes
   just that window and it stays in SBUF (no DRAM corr table, no dma_gather).
 - per-pixel row selection: Pool indirect_copy (13 rows per group-of-16 at the
   group's base row, as 16-elem chunks) + a 2-stage fp16 residual shift ladder.
 - the 6-stage x shift ladder and the x-lerp run BATCHED over all 8 blocks
   (one op per stage, [128, 8, 9, w]) to amortize per-op overhead.
 - everything downstream of PSUM is fp16 (tolerance is 2e-2); lerp weights are
   per-partition f32 scalars (y) / broadcast fp16 planes (x).

Host: sort, f2 window gather, weight/mask/idx precompute, unsort+transpose.
"""

import numpy as np

import concourse.bass as bass
import concourse.mybir as mybir
import concourse.tile as tile
from concourse import bacc
from concourse.bass_utils import run_bass_kernel_spmd

F32 = mybir.dt.float32
F16 = mybir.dt.float16
U16 = mybir.dt.uint16
I16 = mybir.dt.int16
COPY = mybir.ActivationFunctionType.Copy
MULT = mybir.AluOpType.mult
ADD = mybir.AluOpType.add

NCORES = 8
NPC = 1024          # pixels per core
NBLK = 8            # blocks of 128 pixels per core
SROW = 22           # f2 window rows per block (block y-span + 10 <= 22)
NGR = 13            # rows gathered per group of 16 pixels (group span + 10 <= 13)
YSTAGES = [(2, 11), (1, 10)]
XSTAGES = [(32, 41), (16, 25), (8, 17), (4, 13), (2, 11), (1, 10)]
WPAD = 76           # padded row width for the x shift ladder (4 + 64 + 5 + align)

_NC = None


def _sc(st, b, j):
    """[128,1] per-partition scalar view of tile column j, block b."""
    return st[:, b : b + 1, j : j + 1].rearrange("p a c -> p (a c)")


def _build_kernel(tc, out, f1, f2w, wuv, msk, gidx):
    nc = tc.nc
    import contextlib

    with contextlib.ExitStack() as ctx:
        const = ctx.enter_context(tc.tile_pool(name="const", bufs=1))
        work = ctx.enter_context(tc.tile_pool(name="work", bufs=3))
        xwork = ctx.enter_context(tc.tile_pool(name="xwork", bufs=1))
        psum = ctx.enter_context(tc.tile_pool(name="psum", bufs=2, space="PSUM"))

        f1t = const.tile([128, 2, NPC], F16)
        nc.sync.dma_start(f1t[:], f1[:])
        wt_uv = const.tile([128, NBLK, 4], F32)
        nc.sync.dma_start(wt_uv[:], wuv[:])
        mt = const.tile([128, NBLK, 12], F16)
        nc.sync.dma_start(mt[:], msk[:])
        it = const.tile([128, NBLK, 4], U16)
        nc.sync.dma_start(it[:], gidx[:])
        f2t = const.tile([128, NBLK, 2, SROW * 64], F16)

        # batched padded tile for the x ladder; pad columns stay zero
        xp8 = const.tile([128, NBLK, 9, WPAD], F16)
        nc.vector.memset(xp8[:], 0.0)

        for bp in range(4):
            wt2 = work.tile([128, 2, SROW, 64], F16, tag="W")
            gt2 = work.tile([128, 2, NGR, 64], F16, tag="G")
            for h in range(2):
                b = 2 * bp + h
                # ---- stream in this block's f2 window
                nc.sync.dma_start(f2t[:, b, :, :], f2w[:, b, :, :])
                # ---- matmul: W = f1_blk^T @ f2win  (K=256 as 2x128)
                pt = psum.tile([128, SROW * 64], F32, tag="ps")
                lhs = [
                    f1t[:, k : k + 1, b * 128 : (b + 1) * 128].rearrange(
                        "p a c -> p (a c)"
                    )
                    for k in range(2)
                ]
                for n0 in range(0, SROW * 64, 512):
                    n1 = min(n0 + 512, SROW * 64)
                    for k in range(2):
                        nc.tensor.matmul(
                            pt[:, n0:n1],
                            lhsT=lhs[k],
                            rhs=f2t[:, b, k, n0:n1],
                            start=(k == 0),
                            stop=(k == 1),
                        )
                # ---- PSUM -> SBUF fp16 window (split ACT/DVE)
                ptv = pt[:].rearrange("p (r c) -> p r c", c=64)
                nc.scalar.copy(wt2[:, h, 0:11, :], ptv[:, 0:11, :])
                nc.vector.tensor_copy(wt2[:, h, 11:SROW, :], ptv[:, 11:SROW, :])
            # ---- per-16-group row gather (16-elem chunks, 52 per group)
            for h in range(2):
                nc.gpsimd.indirect_copy(
                    gt2[:, h].rearrange("p r (k c) -> p (r k) c", c=16),
                    wt2[:, h].rearrange("p a (k c) -> p (a k) c", c=16),
                    it[:, 2 * bp + h, :],
                    True,
                )
            # ---- paired residual y shift ladder: shift by ry in [0, 3]
            cur = gt2
            for i, (sh, wn) in enumerate(YSTAGES):
                nxt = work.tile([128, 2, wn, 64], F16, tag=f"Y{i}")
                mask = (
                    mt[:, 2 * bp : 2 * bp + 2, i : i + 1]
                    .bitcast(I16)
                    .to_broadcast([128, 2, wn, 64])
                )
                nc.vector.tensor_copy(nxt[:], cur[:, :, 0:wn, :])
                nc.vector.copy_predicated(nxt[:], mask, cur[:, :, sh : sh + wn, :])
                cur = nxt
            # ---- y-lerp into the batched padded tile
            for h in range(2):
                b = 2 * bp + h
                t0 = work.tile([128, 9, 64], F16, tag="T0")
                nc.scalar.activation(
                    t0[:], cur[:, h, 0:9, :], COPY, scale=_sc(wt_uv, b, 0)
                )
                nc.vector.scalar_tensor_tensor(
                    xp8[:, b, :, 4:68],
                    cur[:, h, 1:10, :],
                    _sc(wt_uv, b, 1),
                    t0[:],
                    MULT,
                    ADD,
                )
        # ---- batched x shift ladder over all 8 blocks
        xcur = xp8[:, :, :, 0:73]
        for i, (sh, wn) in enumerate(XSTAGES):
            nxt = xwork.tile([128, NBLK, 9, wn], F16, tag=f"L{i}")
            mask = (
                mt[:, :, 2 + i : 3 + i].bitcast(I16).to_broadcast([128, NBLK, 9, wn])
            )
            nc.vector.tensor_copy(nxt[:], xcur[:, :, :, 0:wn])
            nc.vector.copy_predicated(nxt[:], mask, xcur[:, :, :, sh : sh + wn])
            xcur = nxt
        # ---- batched x-lerp: O = (1-v)X[0:9] + v*X[1:10]
        v1 = mt[:, :, 8:9].to_broadcast([128, NBLK, 9, 9])
        v0 = mt[:, :, 9:10].to_broadcast([128, NBLK, 9, 9])
        ta = xwork.tile([128, NBLK, 9, 9], F16)
        nc.vector.tensor_tensor(ta[:], xcur[:, :, :, 0:9], v1, MULT)
        tb2 = xwork.tile([128, NBLK, 9, 9], F16)
        nc.vector.tensor_tensor(tb2[:], xcur[:, :, :, 1:10], v0, MULT)
        ot8 = xwork.tile([128, NBLK, 9, 9], F32)
        nc.vector.tensor_tensor(ot8[:], ta[:], tb2[:], ADD)
        nc.sync.dma_start(
            out[:].rearrange("(a p) c -> p a c", a=NBLK),
            ot8[:].rearrange("p b a c -> p b (a c)"),
        )


def _build():
    nc = bacc.Bacc("TRN2", target_bir_lowering=False, debug=False)
    f1 = nc.dram_tensor("f1", [128, 2, NPC], F16, kind="ExternalInput").ap()
    f2w = nc.dram_tensor(
        "f2w", [128, NBLK, 2, SROW * 64], F16, kind="ExternalInput"
    ).ap()
    wuv = nc.dram_tensor("wuv", [128, NBLK, 4], F32, kind="ExternalInput").ap()
    msk = nc.dram_tensor("msk", [128, NBLK, 12], F16, kind="ExternalInput").ap()
    gidx = nc.dram_tensor("gidx", [128, NBLK, 4], U16, kind="ExternalInput").ap()
    out = nc.dram_tensor("out", [NPC, 81], F32, kind="ExternalOutput").ap()
    with tile.TileContext(nc) as tc:
        _build_kernel(tc, out, f1, f2w, wuv, msk, gidx)
    nc.compile()
    return nc


def get_nc():
    global _NC
    if _NC is None:
        _NC = _build()
    return _NC


def host_prep(fmap1, fmap2, coords, radius):
    """Per-core input maps. Sorting, window gather, and weights on host."""
    B, D, H, W = fmap1.shape
    assert (B, D, H, W) == (2, 256, 64, 64) and int(radius) == 4
    f1 = (fmap1.reshape(B, D, H * W) / np.float32(16.0)).astype(np.float16)
    f2 = fmap2.reshape(B, D, H, W).astype(np.float16)
    # zero-padded rows: r' = r + 4; extra top slack so base = min(y0) always
    f2p = np.zeros((B, 2, 128, 85, 64), np.float16)
    f2p[:, :, :, 4:68, :] = f2.reshape(B, 2, 128, 64, 64)
    cx = coords[:, 0].reshape(B, H * W).astype(np.float32)
    cy = coords[:, 1].reshape(B, H * W).astype(np.float32)

    in_maps = []
    perms = []
    for c in range(NCORES):
        bb, ps = c // 4, (c % 4) * NPC
        ccx = cx[bb, ps : ps + NPC]
        ccy = cy[bb, ps : ps + NPC]
        y0 = np.floor(ccy).astype(np.int64)  # [0, 63]
        order = np.argsort(y0, kind="stable")
        perms.append(order)
        y0s = y0[order]
        x0s = np.floor(ccx[order]).astype(np.int64)
        us = (ccy[order] - y0s).astype(np.float32)
        vs = (ccx[order] - x0s).astype(np.float32)

        # per-block window bases (padded-row coords), per-group gather bases
        yb = y0s.reshape(NBLK, 128)
        base = yb.min(axis=1)                          # [NBLK], window always fits
        sy = yb - base[:, None]                        # [NBLK, 128]
        assert sy.min() >= 0 and sy.max() <= SROW - 10, (
            f"block y-span too large: {sy.max()}"
        )
        gmin = sy.reshape(NBLK, 8, 16).min(axis=2)     # [NBLK, 8] group base
        assert gmin.max() <= SROW - NGR, f"group base too large: {gmin.max()}"
        ry = sy - np.repeat(gmin, 16, axis=1)          # residual in [0, 3]
        assert ry.max() <= 3, f"group y-span too large: {ry.max()}"

        # f2 windows: [128(K), NBLK, 2(kchunk), SROW*64]
        f2wc = np.empty((2, 128, NBLK, SROW * 64), np.float16)
        for blk in range(NBLK):
            bs = int(base[blk])
            f2wc[:, :, blk, :] = f2p[bb, :, :, bs : bs + SROW, :].reshape(
                2, 128, SROW * 64
            )
        f2wc = np.ascontiguousarray(f2wc.transpose(1, 2, 0, 3))

        # f1 sorted columns: [128(K), 2(kchunk), NPC]
        f1c = np.ascontiguousarray(
            f1[bb][:, ps + order].reshape(2, 128, NPC).transpose(1, 0, 2)
        )

        # per-pixel weights (f32) and ladder masks + x-lerp weights (f16)
        wuvc = np.zeros((128, NBLK, 4), np.float32)
        wuvc[:, :, 0] = (1.0 - us).reshape(NBLK, 128).T
        wuvc[:, :, 1] = us.reshape(NBLK, 128).T
        mskc = np.zeros((128, NBLK, 12), np.float16)
        r = ry.copy()
        for i, (sh, _) in enumerate(YSTAGES):
            bit = (r >= sh).astype(np.int64)
            r = r - bit * sh
            mskc[:, :, i] = bit.astype(np.float16).T
        sx = x0s.reshape(NBLK, 128)
        for i, (sh, _) in enumerate(XSTAGES):
            mskc[:, :, 2 + i] = ((sx // sh) % 2).astype(np.float16).T
        mskc[:, :, 8] = (1.0 - vs).reshape(NBLK, 128).T.astype(np.float16)
        mskc[:, :, 9] = vs.reshape(NBLK, 128).T.astype(np.float16)

        # gather indices: [128, NBLK, 4] uint16; unwrapped chunk i = s*16+j
        # (i < 52): row = i // 4, sub = i % 4 -> (gmin+row)*64 + sub*16
        gidxc = np.zeros((128, NBLK, 4), np.uint16)
        jj = np.tile(np.arange(16), 8)
        gg = np.repeat(np.arange(8), 16)
        for blk in range(NBLK):
            for s in range(4):
                i = np.minimum(s * 16 + jj, 51)
                row = i // 4
                sub = i % 4
                val = (gmin[blk, gg] + row) * 64 + sub * 16
                gidxc[:, blk, s] = val.astype(np.uint16)
        in_maps.append(
            {
                "f1": f1c,
                "f2w": f2wc,
                "wuv": wuvc,
                "msk": np.ascontiguousarray(mskc),
                "gidx": gidxc,
            }
        )
    return in_maps, perms


def assemble(outs, perms):
    """8x [1024, 81] (sorted pixels, k2-major) -> [2, 81, 64, 64], k = k1*9+k2."""
    full = np.empty((NCORES, NPC, 81), np.float32)
    for c in range(NCORES):
        full[c, perms[c]] = outs[c]
    o = full.reshape(2, 4096, 81).reshape(2, 64, 64, 9, 9)
    return np.ascontiguousarray(
        o.transpose(0, 4, 3, 1, 2).reshape(2, 81, 64, 64)
    ).astype(np.float32)


def kernel(**inputs):
    fmap1 = np.asarray(inputs["fmap1"], np.float32)
    fmap2 = np.asarray(inputs["fmap2"], np.float32)
    coords = np.asarray(inputs["coords"], np.float32)
    radius = int(np.asarray(inputs["radius"]))
    in_maps, perms = host_prep(fmap1, fmap2, coords, radius)
    nc = get_nc()
    res = run_bass_kernel_spmd(nc, in_maps, core_ids=list(range(NCORES)))
    return assemble([r["out"] for r in res.results], perms)
